# revision 3
# baseline (speedup 1.0000x reference)
"""Causal multi-head attention (B=2, T=2048, D=1024, H=16) on 8 TRN2 NeuronCores.

Strategy (tensor-parallel over heads + sequence-parallel output projection):
  - Each core owns 2 heads (e-slice of 128 columns of Q/K/V) for BOTH batches.
  - Per core: Q^T/K^T/V^T projections from a replicated transposed input x^T,
    flash-style causal attention computed entirely in the "transposed" layout
    (S^T chunks with k on partitions), softmax without max-subtraction
    (|S/8| < ~15 so exp is safe in fp32/bf16), row sums via a ones-column
    appended to V in the P^T·V matmul.
  - AllToAll over all 8 cores reshards ctx from head-split to row-split
    (each core ends with ctx^T [1024, 512] for its (batch, seq-quarter)).
  - Output projection out[rows, :] = ctx rows @ w_o^T locally per core.
  - Host gathers the 8 disjoint [512, 1024] row blocks.
All matmuls in bf16 (fp32 PSUM accumulation).
"""

import numpy as np
import ml_dtypes

B, T, D, H = 2, 2048, 1024, 16
DH = D // H            # 64
NCORES = 8
ES = 128               # columns of Q/K/V per core (2 heads)
TQ = 512               # q-super width
NTQ = T // TQ          # 4 q-supers per (b, h)
KC = 128               # k-chunk width
NKC = T // KC          # 16 k-chunks
NDC = D // 128         # 8 contraction chunks

_cache = {}


def _build():
    import concourse.bacc as bacc
    import concourse.mybir as mybir
    import concourse.tile as tile

    dt = mybir.dt
    fp32 = dt.float32
    bf16 = dt.bfloat16

    nc = bacc.Bacc("TRN2", target_bir_lowering=False, debug=False,
                   enable_asserts=False, num_devices=NCORES)

    xT_d = nc.dram_tensor("xT", [B, D, T], bf16, kind="ExternalInput")
    wqT_d = nc.dram_tensor("wqT", [D, ES], bf16, kind="ExternalInput")
    wkT_d = nc.dram_tensor("wkT", [D, ES], bf16, kind="ExternalInput")
    wvT_d = nc.dram_tensor("wvT", [D, ES], bf16, kind="ExternalInput")
    woT_d = nc.dram_tensor("woT", [D, D], bf16, kind="ExternalInput")
    masks_d = nc.dram_tensor("masks", [4, KC, TQ], bf16, kind="ExternalInput")
    ident_d = nc.dram_tensor("ident", [128, 128], bf16, kind="ExternalInput")
    out_d = nc.dram_tensor("out", [TQ, D], fp32, kind="ExternalOutput")

    with tile.TileContext(nc) as tc:
        with (
            tc.tile_pool(name="xt", bufs=B * NDC) as xt_pool,
            tc.tile_pool(name="wt", bufs=1) as wt_pool,
            tc.tile_pool(name="qkv", bufs=1) as qkv_pool,
            tc.tile_pool(name="pt", bufs=6) as pt_pool,
            tc.tile_pool(name="sm", bufs=4) as sm_pool,
            tc.tile_pool(name="stage", bufs=4) as stage_pool,
            tc.tile_pool(name="outp", bufs=3) as out_pool,
            tc.tile_pool(name="ps_s", bufs=2, space="PSUM") as ps_s,
            tc.tile_pool(name="ps_pv", bufs=2, space="PSUM") as ps_pv,
            tc.tile_pool(name="ps_mm", bufs=2, space="PSUM") as ps_mm,
            tc.tile_pool(name="ps_misc", bufs=2, space="PSUM") as ps_misc,
            tc.tile_pool(name="dram", bufs=1, space="DRAM") as dram_pool,
        ):
            # ---- constant / weight loads ----
            wq_sb = wt_pool.tile([128, NDC, ES], bf16, name="wq_sb")
            wk_sb = wt_pool.tile([128, NDC, ES], bf16, name="wk_sb")
            wv_sb = wt_pool.tile([128, NDC, ES], bf16, name="wv_sb")
            nc.sync.dma_start(wq_sb[:], wqT_d.rearrange("(c p) e -> p c e", p=128))
            nc.sync.dma_start(wk_sb[:], wkT_d.rearrange("(c p) e -> p c e", p=128))
            nc.sync.dma_start(wv_sb[:], wvT_d.rearrange("(c p) e -> p c e", p=128))
            wo_sb = wt_pool.tile([128, NDC, D], bf16, name="wo_sb")
            nc.sync.dma_start(wo_sb[:], woT_d.rearrange("(c p) e -> p c e", p=128))
            mask_sb = wt_pool.tile([128, 4, TQ], bf16, name="mask_sb")
            nc.sync.dma_start(mask_sb[:], masks_d.rearrange("v p n -> p v n"))
            ident_sb = wt_pool.tile([128, 128], bf16, name="ident_sb")
            nc.sync.dma_start(ident_sb[:], ident_d[:])
            ones_sb = wt_pool.tile([1, 64], bf16, name="ones_sb")
            nc.vector.memset(ones_sb[:], 1.0)

            # x^T tiles: [128, T] per (batch, d-chunk)
            xt = {}
            for b in range(B):
                for c in range(NDC):
                    t_ = xt_pool.tile([128, T], bf16, name=f"xt_{b}_{c}", tag="xt")
                    nc.sync.dma_start(t_[:], xT_d[b, 128 * c:128 * (c + 1), :])
                    xt[(b, c)] = t_

            # ---- projections ----
            # Q^T, K^T: [128 e, B, T];  V^T slab then PE-transpose to natural V
            qt_sb = qkv_pool.tile([128, B, T], bf16, name="qt_sb")
            kt_sb = qkv_pool.tile([128, B, T], bf16, name="kt_sb")
            vt_sb = qkv_pool.tile([128, B, T], bf16, name="vt_sb")
            for b in range(B):
                for s in range(NTQ):
                    for w_sb, dst in ((wq_sb, qt_sb), (wk_sb, kt_sb), (wv_sb, vt_sb)):
                        mm = ps_mm.tile([128, TQ], fp32, name="mm", tag="mm")
                        for c in range(NDC):
                            nc.tensor.matmul(
                                mm[:], w_sb[:, c, :],
                                xt[(b, c)][:, TQ * s: TQ * (s + 1)],
                                start=(c == 0), stop=(c == NDC - 1))
                        nc.scalar.mul(dst[:, b, TQ * s: TQ * (s + 1)], mm[:], 1.0)

            # V natural [128 t, B, k-chunk, 65*2] with ones column per head
            v_sb = qkv_pool.tile([128, B, NKC, 2 * (DH + 1)], bf16, name="v_sb")
            nc.vector.memset(v_sb[:], 1.0)
            for b in range(B):
                for i in range(NKC):
                    tp = ps_misc.tile([128, 128], bf16, name="tp", tag="misc")
                    nc.tensor.transpose(
                        tp[:], vt_sb[:, b, 128 * i: 128 * (i + 1)], ident_sb[:])
                    # write cols {0:64} -> head0 slot, {64:128} -> head1 slot
                    nc.scalar.mul(
                        v_sb[:, b, i, :].rearrange("p (h e) -> p h e", h=2)[:, :, 0:DH],
                        tp.rearrange("p (h e) -> p h e", h=2), 1.0)

            # ---- attention ----
            a2a_in = dram_pool.tile([128, 4096], bf16, name="a2a_in")
            a2a_out = dram_pool.tile([128, 4096], bf16, name="a2a_out")

            for b in range(B):
                for h in range(2):
                    for s in range(NTQ):
                        nj = 4 * s + 4
                        pv = ps_pv.tile([DH + 1, TQ], fp32, name="pv", tag="pv")
                        for j in range(nj):
                            sp = ps_s.tile([128, TQ], fp32, name="sp", tag="sp")
                            nc.tensor.matmul(
                                sp[:],
                                kt_sb[DH * h: DH * (h + 1), b, KC * j: KC * (j + 1)],
                                qt_sb[DH * h: DH * (h + 1), b, TQ * s: TQ * (s + 1)],
                                start=True, stop=True)
                            pt = pt_pool.tile([128, TQ], bf16, name="pt", tag="pt")
                            nc.scalar.activation(
                                pt[:], sp[:], mybir.ActivationFunctionType.Exp,
                                scale=0.125)
                            if j >= 4 * s:
                                v = j - 4 * s
                                nc.vector.tensor_mul(
                                    pt[:], pt[:], mask_sb[:, v, :])
                            nc.tensor.matmul(
                                pv[:],
                                v_sb[:, b, j, (DH + 1) * h: (DH + 1) * (h + 1)],
                                pt[:],
                                start=(j == 0), stop=(j == nj - 1))
                        # normalize: linv = 1/l, broadcast over 64 partitions
                        linv_f = sm_pool.tile([1, TQ], fp32, name="linv_f", tag="lf")
                        nc.vector.reciprocal(linv_f[:], pv[DH:DH + 1, :])
                        linv_b = sm_pool.tile([1, TQ], bf16, name="linv_b", tag="lb")
                        nc.scalar.mul(linv_b[:], linv_f[:], 1.0)
                        bc = ps_misc.tile([DH, TQ], fp32, name="bc", tag="misc")
                        nc.tensor.matmul(bc[:], ones_sb[:], linv_b[:],
                                         start=True, stop=True)
                        bcb = sm_pool.tile([DH, TQ], fp32, name="bcb", tag="bcb")
                        nc.scalar.mul(bcb[:], bc[:], 1.0)
                        ctx = stage_pool.tile([DH, TQ], bf16, name="ctx", tag="ctx")
                        nc.vector.tensor_mul(ctx[:], pv[0:DH, :], bcb[:])
                        # stage into a2a_in: shard dst j_dst = 4b + s,
                        # partition p of the [128,512] chunk lives at
                        # row 16*j_dst + p//8, col (p%8)*512; this head is
                        # partitions 64h..64h+63 -> rows 16*j_dst+8h ..+8
                        j_dst = 4 * b + s
                        r0 = 16 * j_dst + 8 * h
                        nc.sync.dma_start(
                            a2a_in[r0: r0 + 8, :].rearrange(
                                "r (s2 n) -> (r s2) n", s2=8),
                            ctx[:])

            # ---- all-to-all reshard ----
            nc.gpsimd.collective_compute(
                "AllToAll", mybir.AluOpType.bypass,
                replica_groups=[list(range(NCORES))],
                ins=[a2a_in.opt()], outs=[a2a_out.opt()])

            # gather received shards: ctx^T full [1024, 512] as [128, 8, 512]
            ctxf = qkv_pool.tile([128, NDC, TQ], bf16, name="ctxf")
            for j in range(NDC):
                nc.sync.dma_start(
                    ctxf[:, j, :],
                    a2a_out[16 * j: 16 * (j + 1), :].rearrange(
                        "r (s2 n) -> (r s2) n", s2=8))

            # ---- output projection: out[128i:.., 512n:..] ----
            for i in range(TQ // 128):
                for n in range(2):
                    mm = ps_mm.tile([128, TQ], fp32, name="mm", tag="mm")
                    for j in range(NDC):
                        nc.tensor.matmul(
                            mm[:], ctxf[:, j, 128 * i: 128 * (i + 1)],
                            wo_sb[:, j, TQ * n: TQ * (n + 1)],
                            start=(j == 0), stop=(j == NDC - 1))
                    ob = out_pool.tile([128, TQ], fp32, name="ob", tag="ob")
                    nc.scalar.mul(ob[:], mm[:], 1.0)
                    nc.sync.dma_start(
                        out_d[128 * i: 128 * (i + 1), TQ * n: TQ * (n + 1)], ob[:])

    nc.compile()
    return nc


def _prep_inputs(x, w_q, w_k, w_v, w_o):
    bf = ml_dtypes.bfloat16
    xT = np.ascontiguousarray(np.transpose(np.asarray(x, np.float32),
                                           (0, 2, 1))).astype(bf)
    woT = np.ascontiguousarray(np.asarray(w_o, np.float32).T).astype(bf)
    masks = np.zeros((4, KC, TQ), dtype=bf)
    ql = np.arange(TQ)[None, :]
    kl = np.arange(KC)[:, None]
    for v in range(4):
        masks[v] = (ql >= kl + 128 * v).astype(bf)
    ident = np.eye(128, dtype=bf)
    in_maps = []
    for c in range(NCORES):
        sl = slice(ES * c, ES * (c + 1))
        in_maps.append({
            "xT": xT,
            "wqT": np.ascontiguousarray(np.asarray(w_q, np.float32)[sl, :].T).astype(bf),
            "wkT": np.ascontiguousarray(np.asarray(w_k, np.float32)[sl, :].T).astype(bf),
            "wvT": np.ascontiguousarray(np.asarray(w_v, np.float32)[sl, :].T).astype(bf),
            "woT": woT,
            "masks": masks,
            "ident": ident,
        })
    return in_maps


def kernel(x, w_q, w_k, w_v, w_o, _run_kwargs=None):
    from concourse.bass_utils import run_bass_kernel_spmd
    if "nc" not in _cache:
        _cache["nc"] = _build()
    nc = _cache["nc"]
    in_maps = _prep_inputs(x, w_q, w_k, w_v, w_o)
    res = run_bass_kernel_spmd(nc, in_maps, core_ids=list(range(NCORES)),
                               **(_run_kwargs or {}))
    _cache["last_result"] = res
    out = np.empty((B, T, D), dtype=np.float32)
    for c in range(NCORES):
        b, s = c // 4, c % 4
        out[b, TQ * s: TQ * (s + 1), :] = res.results[c]["out"]
    return out


# revision 13
# speedup vs baseline: 1.3505x; 1.3505x over previous
"""Causal multi-head attention (B=2, T=2048, D=1024, H=16) on 8 TRN2 NeuronCores.

Strategy (tensor-parallel over heads + sequence-parallel output projection):
  - Each core owns 2 heads (e-slice of 128 columns of Q/K/V) for BOTH batches.
  - Per core: Q^T/K^T/V^T projections from a replicated transposed input x^T,
    flash-style causal attention computed entirely in the "transposed" layout
    (S^T chunks with k on partitions), softmax without max-subtraction
    (|S/8| < ~15 so exp is safe in fp32/bf16), row sums via a ones-column
    appended to V in the P^T.V matmul.
  - Two AllToAlls over all 8 cores (one per local head, the first overlapped
    with the second head's attention) reshard ctx from head-split to row-split
    (each core ends with ctx^T [1024, 512] for its (batch, seq-quarter)).
  - Output projection out[rows, :] = ctx rows @ w_o^T locally per core.
  - Host gathers the 8 disjoint [512, 1024] row blocks.
All matmuls in bf16 (fp32 PSUM accumulation).
"""

import numpy as np
import ml_dtypes

B, T, D, H = 2, 2048, 1024, 16
DH = D // H            # 64
NCORES = 8
ES = 128               # columns of Q/K/V per core (2 heads)
TQ = 512               # q-super width
NTQ = T // TQ          # 4 q-supers per (b, h)
KC = 128               # k-chunk width
NKC = T // KC          # 16 k-chunks
NDC = D // 128         # 8 contraction chunks

_cache = {}

PS_S_BUFS = 3
PS_PV_BUFS = 2
PS_MM_BUFS = 2
PS_MISC_BUFS = 1


def _build(reps=1):
    import concourse.bacc as bacc
    import concourse.mybir as mybir
    import concourse.tile as tile

    dt = mybir.dt
    fp32 = dt.float32
    bf16 = dt.bfloat16

    nc = bacc.Bacc("TRN2", target_bir_lowering=False, debug=False,
                   enable_asserts=False, num_devices=NCORES)

    xT_d = nc.dram_tensor("xT", [B, D, T], bf16, kind="ExternalInput")
    wqT_d = nc.dram_tensor("wqT", [D, ES], bf16, kind="ExternalInput")
    wkT_d = nc.dram_tensor("wkT", [D, ES], bf16, kind="ExternalInput")
    wvT_d = nc.dram_tensor("wvT", [D, ES], bf16, kind="ExternalInput")
    woT_d = nc.dram_tensor("woT", [D, D], bf16, kind="ExternalInput")
    masks_d = nc.dram_tensor("masks", [4, KC, TQ], bf16, kind="ExternalInput")
    ident_d = nc.dram_tensor("ident", [128, 128], bf16, kind="ExternalInput")
    out_d = nc.dram_tensor("out", [TQ, D], fp32, kind="ExternalOutput")

    with tile.TileContext(nc) as tc:
        with (
            tc.tile_pool(name="xt", bufs=B * NDC) as xt_pool,
            tc.tile_pool(name="wt", bufs=1) as wt_pool,
            tc.tile_pool(name="qkv", bufs=1) as qkv_pool,
            tc.tile_pool(name="pt", bufs=6) as pt_pool,
            tc.tile_pool(name="sm", bufs=4) as sm_pool,
            tc.tile_pool(name="stage", bufs=4) as stage_pool,
            tc.tile_pool(name="outp", bufs=3) as out_pool,
            tc.tile_pool(name="ps_s", bufs=PS_S_BUFS, space="PSUM") as ps_s,
            tc.tile_pool(name="ps_pv", bufs=PS_PV_BUFS, space="PSUM") as ps_pv,
            tc.tile_pool(name="ps_mm", bufs=PS_MM_BUFS, space="PSUM") as ps_mm,
            tc.tile_pool(name="ps_misc", bufs=PS_MISC_BUFS, space="PSUM") as ps_misc,
            tc.tile_pool(name="dram", bufs=2, space="DRAM") as dram_pool,
        ):
            for rep in range(reps):
                _emit(nc, tc, mybir, fp32, bf16,
                      xT_d, wqT_d, wkT_d, wvT_d, woT_d, masks_d, ident_d, out_d,
                      xt_pool, wt_pool, qkv_pool, pt_pool, sm_pool, stage_pool,
                      out_pool, ps_s, ps_pv, ps_mm, ps_misc, dram_pool)

    nc.compile()
    return nc


def _emit(nc, tc, mybir, fp32, bf16,
          xT_d, wqT_d, wkT_d, wvT_d, woT_d, masks_d, ident_d, out_d,
          xt_pool, wt_pool, qkv_pool, pt_pool, sm_pool, stage_pool,
          out_pool, ps_s, ps_pv, ps_mm, ps_misc, dram_pool):
    # ---- constant / weight loads ----
    wq_sb = wt_pool.tile([128, NDC, ES], bf16, name="wq_sb", tag="wq")
    wk_sb = wt_pool.tile([128, NDC, ES], bf16, name="wk_sb", tag="wk")
    wv_sb = wt_pool.tile([128, NDC, ES], bf16, name="wv_sb", tag="wv")
    nc.sync.dma_start(wq_sb[:], wqT_d.rearrange("(c p) e -> p c e", p=128))
    nc.sync.dma_start(wk_sb[:], wkT_d.rearrange("(c p) e -> p c e", p=128))
    nc.sync.dma_start(wv_sb[:], wvT_d.rearrange("(c p) e -> p c e", p=128))
    wo_sb = wt_pool.tile([128, NDC, D], bf16, name="wo_sb", tag="wo")
    nc.sync.dma_start(wo_sb[:], woT_d.rearrange("(c p) e -> p c e", p=128))
    mask_sb = wt_pool.tile([128, 4, TQ], bf16, name="mask_sb", tag="mask")
    nc.sync.dma_start(mask_sb[:], masks_d.rearrange("v p n -> p v n"))
    ident_sb = wt_pool.tile([128, 128], bf16, name="ident_sb", tag="ident")
    nc.sync.dma_start(ident_sb[:], ident_d[:])
    ones_sb = wt_pool.tile([1, 64], bf16, name="ones_sb", tag="ones")
    nc.vector.memset(ones_sb[:], 1.0)

    # ---- per-batch pipeline: xT DMA -> projections (V transposed inline)
    # -> h=0 attention, so batch 1's projections overlap batch 0's attention.
    qt_sb = qkv_pool.tile([128, B, T], bf16, name="qt_sb", tag="qt")
    kt_sb = qkv_pool.tile([128, B, T], bf16, name="kt_sb", tag="kt")
    vt_sb = qkv_pool.tile([128, B, T], bf16, name="vt_sb", tag="vt")
    v_sb = qkv_pool.tile([128, B, NKC, 2 * (DH + 1)], bf16, name="v_sb", tag="v")
    nc.vector.memset(v_sb[:], 1.0)

    # a2a buffer layout per h: [64 rows, 4096 bf16]; shard j = 8 rows.
    # partition p (0..63) of the [64, 512] (b, s)-chunk lives at
    # row 8*j_dst + p//8, col (p%8)*512, where j_dst = 4b + s.
    a2a_in = {}
    a2a_out = {}
    for h in range(2):
        a2a_in[h] = dram_pool.tile([64, 4096], bf16, name=f"a2a_in{h}",
                                   tag=f"a2a_in{h}")
        a2a_out[h] = dram_pool.tile([64, 4096], bf16, name=f"a2a_out{h}",
                                    tag=f"a2a_out{h}")

    xt = {}

    def emit_xt(b):
        # x^T tiles [128, T] per d-chunk; DMA in half-tiles, half-major, so
        # the first t-supers' projections start after ~2MB instead of ~4MB
        for c in range(NDC):
            xt[(b, c)] = xt_pool.tile([128, T], bf16, name=f"xt_{b}_{c}",
                                      tag="xt")
        for half in range(2):
            for c in range(NDC):
                nc.sync.dma_start(
                    xt[(b, c)][:, 1024 * half: 1024 * (half + 1)],
                    xT_d[b, 128 * c:128 * (c + 1),
                         1024 * half: 1024 * (half + 1)])

    def emit_proj(b, s):
        for w_sb, dst in ((wq_sb, qt_sb), (wk_sb, kt_sb), (wv_sb, vt_sb)):
            mm = ps_mm.tile([128, TQ], fp32, name="mm", tag="mm")
            for c in range(NDC):
                nc.tensor.matmul(
                    mm[:], w_sb[:, c, :],
                    xt[(b, c)][:, TQ * s: TQ * (s + 1)],
                    start=(c == 0), stop=(c == NDC - 1))
            nc.vector.tensor_copy(dst[:, b, TQ * s: TQ * (s + 1)], mm[:])
        # V natural for this t-super's 4 k-chunks, with ones columns
        for i in range(4 * s, 4 * s + 4):
            tp = ps_misc.tile([128, 128], bf16, name="tp", tag="misc")
            nc.tensor.transpose(
                tp[:], vt_sb[:, b, 128 * i: 128 * (i + 1)], ident_sb[:])
            nc.vector.tensor_copy(
                v_sb[:, b, i, :].rearrange("p (h e) -> p h e", h=2)[:, :, 0:DH],
                tp.rearrange("p (h e) -> p h e", h=2))

    def emit_attn(h, b, s_list=None):
        for s in (range(NTQ) if s_list is None else s_list):
            nj = 4 * s + 4
            pv = ps_pv.tile([DH + 1, TQ], fp32, name="pv", tag="pv")
            for j in range(nj):
                v_ = j - 4 * s
                q0 = 128 * v_ if v_ > 0 else 0   # valid q starts here
                sp = ps_s.tile([128, TQ], fp32, name="sp", tag="sp")
                nc.tensor.matmul(
                    sp[:, q0:],
                    kt_sb[DH * h: DH * (h + 1), b, KC * j: KC * (j + 1)],
                    qt_sb[DH * h: DH * (h + 1), b, TQ * s + q0: TQ * (s + 1)],
                    start=True, stop=True)
                pt = pt_pool.tile([128, TQ], bf16, name="pt", tag="pt")
                nc.scalar.activation(
                    pt[:, q0:], sp[:, q0:],
                    mybir.ActivationFunctionType.Exp, scale=0.125)
                if v_ >= 0:
                    nc.vector.tensor_mul(
                        pt[:, q0:], pt[:, q0:], mask_sb[:, v_, q0:])
                nc.tensor.matmul(
                    pv[:, q0:],
                    v_sb[:, b, j, (DH + 1) * h: (DH + 1) * (h + 1)],
                    pt[:, q0:],
                    start=(j == 0), stop=(j == nj - 1))
            # normalize: linv = 1/l broadcast over 64 partitions via PE
            linv_f = sm_pool.tile([1, TQ], fp32, name="linv_f", tag="lf")
            nc.vector.reciprocal(linv_f[:], pv[DH:DH + 1, :])
            linv_b = sm_pool.tile([1, TQ], bf16, name="linv_b", tag="lb")
            nc.vector.tensor_copy(linv_b[:], linv_f[:])
            bc = ps_misc.tile([DH, TQ], fp32, name="bc", tag="misc")
            nc.tensor.matmul(bc[:], ones_sb[:], linv_b[:],
                             start=True, stop=True)
            bcb = sm_pool.tile([DH, TQ], fp32, name="bcb", tag="bcb")
            nc.vector.tensor_copy(bcb[:], bc[:])
            ctx = stage_pool.tile([DH, TQ], bf16, name="ctx", tag="ctx")
            nc.vector.tensor_mul(ctx[:], pv[0:DH, :], bcb[:])
            j_dst = 4 * b + s
            r0 = 8 * j_dst
            nc.sync.dma_start(
                a2a_in[h][r0: r0 + 8, :].rearrange(
                    "r (s2 n) -> (r s2) n", s2=8),
                ctx[:])

    def emit_a2a(h):
        nc.gpsimd.collective_compute(
            "AllToAll", mybir.AluOpType.bypass,
            replica_groups=[list(range(NCORES))],
            ins=[a2a_in[h].opt()], outs=[a2a_out[h].opt()])

    # pipeline: interleave h=0 attention with projections at t-super
    # granularity so the exp stream starts as early as possible; batch-1
    # projections overlap batch-0 attention; first A2A (h=0 data) overlaps
    # second-head attention.
    emit_xt(0)
    for s in range(NTQ):
        emit_proj(0, s)
        emit_attn(0, 0, [s])
    emit_xt(1)
    for s in range(NTQ):
        emit_proj(1, s)
        emit_attn(0, 1, [s])
    emit_a2a(0)
    emit_attn(1, 0)
    emit_attn(1, 1)

    # gather received shards: ctx^T full [1024, 512] as [128, 8, 512];
    # e-chunk j rows 0:64 = head 2j (h=0 a2a), rows 64:128 = head 2j+1.
    emit_a2a(1)
    ctxf = qkv_pool.tile([128, NDC, TQ], bf16, name="ctxf", tag="ctxf")
    for h in range(2):
        for j in range(NDC):
            nc.sync.dma_start(
                ctxf[64 * h: 64 * (h + 1), j, :],
                a2a_out[h][8 * j: 8 * (j + 1), :].rearrange(
                    "r (s2 n) -> (r s2) n", s2=8))

    # ---- output projection ----
    for i in range(TQ // 128):
        for n in range(2):
            mm = ps_mm.tile([128, TQ], fp32, name="mm", tag="mm")
            for j in range(NDC):
                nc.tensor.matmul(
                    mm[:], ctxf[:, j, 128 * i: 128 * (i + 1)],
                    wo_sb[:, j, TQ * n: TQ * (n + 1)],
                    start=(j == 0), stop=(j == NDC - 1))
            ob = out_pool.tile([128, TQ], fp32, name="ob", tag="ob")
            nc.vector.tensor_copy(ob[:], mm[:])
            nc.sync.dma_start(
                out_d[128 * i: 128 * (i + 1), TQ * n: TQ * (n + 1)], ob[:])


def _prep_inputs(x, w_q, w_k, w_v, w_o):
    bf = ml_dtypes.bfloat16
    xT = np.ascontiguousarray(np.transpose(np.asarray(x, np.float32),
                                           (0, 2, 1))).astype(bf)
    woT = np.ascontiguousarray(np.asarray(w_o, np.float32).T).astype(bf)
    masks = np.zeros((4, KC, TQ), dtype=bf)
    ql = np.arange(TQ)[None, :]
    kl = np.arange(KC)[:, None]
    for v in range(4):
        masks[v] = (ql >= kl + 128 * v).astype(bf)
    ident = np.eye(128, dtype=bf)
    in_maps = []
    for c in range(NCORES):
        # heads (2c, 2c+1): h=0 slice rows [128c, 128c+64), h=1 the next 64
        sl = slice(ES * c, ES * (c + 1))
        in_maps.append({
            "xT": xT,
            "wqT": np.ascontiguousarray(np.asarray(w_q, np.float32)[sl, :].T).astype(bf),
            "wkT": np.ascontiguousarray(np.asarray(w_k, np.float32)[sl, :].T).astype(bf),
            "wvT": np.ascontiguousarray(np.asarray(w_v, np.float32)[sl, :].T).astype(bf),
            "woT": woT,
            "masks": masks,
            "ident": ident,
        })
    return in_maps


def kernel(x, w_q, w_k, w_v, w_o, _run_kwargs=None):
    from concourse.bass_utils import run_bass_kernel_spmd
    if "nc" not in _cache:
        _cache["nc"] = _build()
    nc = _cache["nc"]
    in_maps = _prep_inputs(x, w_q, w_k, w_v, w_o)
    res = run_bass_kernel_spmd(nc, in_maps, core_ids=list(range(NCORES)),
                               **(_run_kwargs or {}))
    _cache["last_result"] = res
    out = np.empty((B, T, D), dtype=np.float32)
    for c in range(NCORES):
        b, s = c // 4, c % 4
        out[b, TQ * s: TQ * (s + 1), :] = res.results[c]["out"]
    return out


# revision 14
# speedup vs baseline: 1.4055x; 1.0407x over previous
"""Causal multi-head attention (B=2, T=2048, D=1024, H=16) on 8 TRN2 NeuronCores.

Strategy (tensor-parallel over heads + sequence-parallel output projection):
  - Each core owns 2 heads (e-slice of 128 columns of Q/K/V) for BOTH batches.
  - Per core: Q^T/K^T/V^T projections from a replicated transposed input x^T,
    flash-style causal attention computed entirely in the "transposed" layout
    (S^T chunks with k on partitions), softmax without max-subtraction
    (|S/8| < ~15 so exp is safe in fp32/bf16), row sums via a ones-column
    appended to V in the P^T.V matmul.
  - Two AllToAlls over all 8 cores (one per local head, the first overlapped
    with the second head's attention) reshard ctx from head-split to row-split
    (each core ends with ctx^T [1024, 512] for its (batch, seq-quarter)).
  - Output projection out[rows, :] = ctx rows @ w_o^T locally per core.
  - Host gathers the 8 disjoint [512, 1024] row blocks.
All matmuls in bf16 (fp32 PSUM accumulation).
"""

import numpy as np
import ml_dtypes

B, T, D, H = 2, 2048, 1024, 16
DH = D // H            # 64
NCORES = 8
ES = 128               # columns of Q/K/V per core (2 heads)
TQ = 512               # q-super width
NTQ = T // TQ          # 4 q-supers per (b, h)
KC = 128               # k-chunk width
NKC = T // KC          # 16 k-chunks
NDC = D // 128         # 8 contraction chunks

_cache = {}

PS_S_BUFS = 3
PS_PV_BUFS = 2
PS_MM_BUFS = 2
PS_MISC_BUFS = 1


def _build(reps=1):
    import concourse.bacc as bacc
    import concourse.mybir as mybir
    import concourse.tile as tile

    dt = mybir.dt
    fp32 = dt.float32
    bf16 = dt.bfloat16

    nc = bacc.Bacc("TRN2", target_bir_lowering=False, debug=False,
                   enable_asserts=False, num_devices=NCORES)

    xT_d = nc.dram_tensor("xT", [B, D, T], bf16, kind="ExternalInput")
    wqT_d = nc.dram_tensor("wqT", [D, ES], bf16, kind="ExternalInput")
    wkT_d = nc.dram_tensor("wkT", [D, ES], bf16, kind="ExternalInput")
    wvT_d = nc.dram_tensor("wvT", [D, ES], bf16, kind="ExternalInput")
    woT_d = nc.dram_tensor("woT", [D, D], bf16, kind="ExternalInput")
    masks_d = nc.dram_tensor("masks", [4, KC, TQ], bf16, kind="ExternalInput")
    ident_d = nc.dram_tensor("ident", [128, 128], bf16, kind="ExternalInput")
    out_d = nc.dram_tensor("out", [TQ, D], fp32, kind="ExternalOutput")

    with tile.TileContext(nc) as tc:
        with (
            tc.tile_pool(name="xt", bufs=B * NDC) as xt_pool,
            tc.tile_pool(name="wt", bufs=1) as wt_pool,
            tc.tile_pool(name="qkv", bufs=1) as qkv_pool,
            tc.tile_pool(name="pt", bufs=6) as pt_pool,
            tc.tile_pool(name="sm", bufs=4) as sm_pool,
            tc.tile_pool(name="stage", bufs=4) as stage_pool,
            tc.tile_pool(name="outp", bufs=3) as out_pool,
            tc.tile_pool(name="ps_s", bufs=PS_S_BUFS, space="PSUM") as ps_s,
            tc.tile_pool(name="ps_pv", bufs=PS_PV_BUFS, space="PSUM") as ps_pv,
            tc.tile_pool(name="ps_mm", bufs=PS_MM_BUFS, space="PSUM") as ps_mm,
            tc.tile_pool(name="ps_misc", bufs=PS_MISC_BUFS, space="PSUM") as ps_misc,
            tc.tile_pool(name="dram", bufs=2, space="DRAM") as dram_pool,
        ):
            for rep in range(reps):
                _emit(nc, tc, mybir, fp32, bf16,
                      xT_d, wqT_d, wkT_d, wvT_d, woT_d, masks_d, ident_d, out_d,
                      xt_pool, wt_pool, qkv_pool, pt_pool, sm_pool, stage_pool,
                      out_pool, ps_s, ps_pv, ps_mm, ps_misc, dram_pool)

    nc.compile()
    return nc


def _emit(nc, tc, mybir, fp32, bf16,
          xT_d, wqT_d, wkT_d, wvT_d, woT_d, masks_d, ident_d, out_d,
          xt_pool, wt_pool, qkv_pool, pt_pool, sm_pool, stage_pool,
          out_pool, ps_s, ps_pv, ps_mm, ps_misc, dram_pool):
    # ---- constant / weight loads ----
    wq_sb = wt_pool.tile([128, NDC, ES], bf16, name="wq_sb", tag="wq")
    wk_sb = wt_pool.tile([128, NDC, ES], bf16, name="wk_sb", tag="wk")
    wv_sb = wt_pool.tile([128, NDC, ES], bf16, name="wv_sb", tag="wv")
    nc.sync.dma_start(wq_sb[:], wqT_d.rearrange("(c p) e -> p c e", p=128))
    nc.sync.dma_start(wk_sb[:], wkT_d.rearrange("(c p) e -> p c e", p=128))
    nc.sync.dma_start(wv_sb[:], wvT_d.rearrange("(c p) e -> p c e", p=128))
    wo_sb = wt_pool.tile([128, NDC, D], bf16, name="wo_sb", tag="wo")
    mask_sb = wt_pool.tile([128, 4, TQ], bf16, name="mask_sb", tag="mask")
    ident_sb = wt_pool.tile([128, 128], bf16, name="ident_sb", tag="ident")
    ones_sb = wt_pool.tile([1, 64], bf16, name="ones_sb", tag="ones")
    nc.vector.memset(ones_sb[:], 1.0)

    # ---- per-batch pipeline: xT DMA -> projections (V transposed inline)
    # -> h=0 attention, so batch 1's projections overlap batch 0's attention.
    qt_sb = qkv_pool.tile([128, B, T], bf16, name="qt_sb", tag="qt")
    kt_sb = qkv_pool.tile([128, B, T], bf16, name="kt_sb", tag="kt")
    vt_sb = qkv_pool.tile([128, B, T], bf16, name="vt_sb", tag="vt")
    v_sb = qkv_pool.tile([128, B, NKC, 2 * (DH + 1)], bf16, name="v_sb", tag="v")
    nc.vector.memset(v_sb[:], 1.0)

    # a2a buffer layout per h: [64 rows, 4096 bf16]; shard j = 8 rows.
    # partition p (0..63) of the [64, 512] (b, s)-chunk lives at
    # row 8*j_dst + p//8, col (p%8)*512, where j_dst = 4b + s.
    a2a_in = {}
    a2a_out = {}
    for h in range(2):
        a2a_in[h] = dram_pool.tile([64, 4096], bf16, name=f"a2a_in{h}",
                                   tag=f"a2a_in{h}")
        a2a_out[h] = dram_pool.tile([64, 4096], bf16, name=f"a2a_out{h}",
                                    tag=f"a2a_out{h}")

    xt = {}

    def emit_xt(b):
        # x^T tiles [128, T] per d-chunk; DMA in half-tiles, half-major, so
        # the first t-supers' projections start after ~2MB instead of ~4MB
        for c in range(NDC):
            xt[(b, c)] = xt_pool.tile([128, T], bf16, name=f"xt_{b}_{c}",
                                      tag="xt")
        for half in range(2):
            for c in range(NDC):
                nc.sync.dma_start(
                    xt[(b, c)][:, 1024 * half: 1024 * (half + 1)],
                    xT_d[b, 128 * c:128 * (c + 1),
                         1024 * half: 1024 * (half + 1)])

    def emit_proj(b, s):
        for w_sb, dst in ((wq_sb, qt_sb), (wk_sb, kt_sb), (wv_sb, vt_sb)):
            mm = ps_mm.tile([128, TQ], fp32, name="mm", tag="mm")
            for c in range(NDC):
                nc.tensor.matmul(
                    mm[:], w_sb[:, c, :],
                    xt[(b, c)][:, TQ * s: TQ * (s + 1)],
                    start=(c == 0), stop=(c == NDC - 1))
            nc.vector.tensor_copy(dst[:, b, TQ * s: TQ * (s + 1)], mm[:])
        # V natural for this t-super's 4 k-chunks, with ones columns
        for i in range(4 * s, 4 * s + 4):
            tp = ps_misc.tile([128, 128], bf16, name="tp", tag="misc")
            nc.tensor.transpose(
                tp[:], vt_sb[:, b, 128 * i: 128 * (i + 1)], ident_sb[:])
            nc.vector.tensor_copy(
                v_sb[:, b, i, :].rearrange("p (h e) -> p h e", h=2)[:, :, 0:DH],
                tp.rearrange("p (h e) -> p h e", h=2))

    def emit_attn(h, b, s_list=None):
        for s in (range(NTQ) if s_list is None else s_list):
            nj = 4 * s + 4
            pv = ps_pv.tile([DH + 1, TQ], fp32, name="pv", tag="pv")
            for j in range(nj):
                v_ = j - 4 * s
                q0 = 128 * v_ if v_ > 0 else 0   # valid q starts here
                sp = ps_s.tile([128, TQ], fp32, name="sp", tag="sp")
                nc.tensor.matmul(
                    sp[:, q0:],
                    kt_sb[DH * h: DH * (h + 1), b, KC * j: KC * (j + 1)],
                    qt_sb[DH * h: DH * (h + 1), b, TQ * s + q0: TQ * (s + 1)],
                    start=True, stop=True)
                pt = pt_pool.tile([128, TQ], bf16, name="pt", tag="pt")
                nc.scalar.activation(
                    pt[:, q0:], sp[:, q0:],
                    mybir.ActivationFunctionType.Exp, scale=0.125)
                if v_ >= 0:
                    nc.vector.tensor_mul(
                        pt[:, q0:], pt[:, q0:], mask_sb[:, v_, q0:])
                nc.tensor.matmul(
                    pv[:, q0:],
                    v_sb[:, b, j, (DH + 1) * h: (DH + 1) * (h + 1)],
                    pt[:, q0:],
                    start=(j == 0), stop=(j == nj - 1))
            # normalize: linv = 1/l broadcast over 64 partitions via PE
            linv_f = sm_pool.tile([1, TQ], fp32, name="linv_f", tag="lf")
            nc.vector.reciprocal(linv_f[:], pv[DH:DH + 1, :])
            linv_b = sm_pool.tile([1, TQ], bf16, name="linv_b", tag="lb")
            nc.vector.tensor_copy(linv_b[:], linv_f[:])
            bc = ps_misc.tile([DH, TQ], fp32, name="bc", tag="misc")
            nc.tensor.matmul(bc[:], ones_sb[:], linv_b[:],
                             start=True, stop=True)
            bcb = sm_pool.tile([DH, TQ], fp32, name="bcb", tag="bcb")
            nc.vector.tensor_copy(bcb[:], bc[:])
            ctx = stage_pool.tile([DH, TQ], bf16, name="ctx", tag="ctx")
            nc.vector.tensor_mul(ctx[:], pv[0:DH, :], bcb[:])
            j_dst = 4 * b + s
            r0 = 8 * j_dst
            nc.sync.dma_start(
                a2a_in[h][r0: r0 + 8, :].rearrange(
                    "r (s2 n) -> (r s2) n", s2=8),
                ctx[:])

    def emit_a2a(h):
        nc.gpsimd.collective_compute(
            "AllToAll", mybir.AluOpType.bypass,
            replica_groups=[list(range(NCORES))],
            ins=[a2a_in[h].opt()], outs=[a2a_out[h].opt()])

    # pipeline: interleave h=0 attention with projections at t-super
    # granularity so the exp stream starts as early as possible; batch-1
    # projections overlap batch-0 attention; first A2A (h=0 data) overlaps
    # second-head attention.
    emit_xt(0)
    nc.sync.dma_start(ident_sb[:], ident_d[:])
    nc.sync.dma_start(mask_sb[:], masks_d.rearrange("v p n -> p v n"))
    for s in range(NTQ):
        emit_proj(0, s)
        emit_attn(0, 0, [s])
    emit_xt(1)
    nc.sync.dma_start(wo_sb[:], woT_d.rearrange("(c p) e -> p c e", p=128))
    for s in range(NTQ):
        emit_proj(1, s)
        emit_attn(0, 1, [s])
    emit_a2a(0)
    emit_attn(1, 0)
    emit_attn(1, 1)

    # gather received shards: ctx^T full [1024, 512] as [128, 8, 512];
    # e-chunk j rows 0:64 = head 2j (h=0 a2a), rows 64:128 = head 2j+1.
    emit_a2a(1)
    ctxf = qkv_pool.tile([128, NDC, TQ], bf16, name="ctxf", tag="ctxf")
    for h in range(2):
        for j in range(NDC):
            nc.sync.dma_start(
                ctxf[64 * h: 64 * (h + 1), j, :],
                a2a_out[h][8 * j: 8 * (j + 1), :].rearrange(
                    "r (s2 n) -> (r s2) n", s2=8))

    # ---- output projection ----
    for i in range(TQ // 128):
        for n in range(2):
            mm = ps_mm.tile([128, TQ], fp32, name="mm", tag="mm")
            for j in range(NDC):
                nc.tensor.matmul(
                    mm[:], ctxf[:, j, 128 * i: 128 * (i + 1)],
                    wo_sb[:, j, TQ * n: TQ * (n + 1)],
                    start=(j == 0), stop=(j == NDC - 1))
            ob = out_pool.tile([128, TQ], fp32, name="ob", tag="ob")
            nc.vector.tensor_copy(ob[:], mm[:])
            nc.sync.dma_start(
                out_d[128 * i: 128 * (i + 1), TQ * n: TQ * (n + 1)], ob[:])


def _prep_inputs(x, w_q, w_k, w_v, w_o):
    bf = ml_dtypes.bfloat16
    xT = np.ascontiguousarray(np.transpose(np.asarray(x, np.float32),
                                           (0, 2, 1))).astype(bf)
    woT = np.ascontiguousarray(np.asarray(w_o, np.float32).T).astype(bf)
    masks = np.zeros((4, KC, TQ), dtype=bf)
    ql = np.arange(TQ)[None, :]
    kl = np.arange(KC)[:, None]
    for v in range(4):
        masks[v] = (ql >= kl + 128 * v).astype(bf)
    ident = np.eye(128, dtype=bf)
    in_maps = []
    for c in range(NCORES):
        # heads (2c, 2c+1): h=0 slice rows [128c, 128c+64), h=1 the next 64
        sl = slice(ES * c, ES * (c + 1))
        in_maps.append({
            "xT": xT,
            "wqT": np.ascontiguousarray(np.asarray(w_q, np.float32)[sl, :].T).astype(bf),
            "wkT": np.ascontiguousarray(np.asarray(w_k, np.float32)[sl, :].T).astype(bf),
            "wvT": np.ascontiguousarray(np.asarray(w_v, np.float32)[sl, :].T).astype(bf),
            "woT": woT,
            "masks": masks,
            "ident": ident,
        })
    return in_maps


def kernel(x, w_q, w_k, w_v, w_o, _run_kwargs=None):
    from concourse.bass_utils import run_bass_kernel_spmd
    if "nc" not in _cache:
        _cache["nc"] = _build()
    nc = _cache["nc"]
    in_maps = _prep_inputs(x, w_q, w_k, w_v, w_o)
    res = run_bass_kernel_spmd(nc, in_maps, core_ids=list(range(NCORES)),
                               **(_run_kwargs or {}))
    _cache["last_result"] = res
    out = np.empty((B, T, D), dtype=np.float32)
    for c in range(NCORES):
        b, s = c // 4, c % 4
        out[b, TQ * s: TQ * (s + 1), :] = res.results[c]["out"]
    return out


# revision 16
# speedup vs baseline: 1.4072x; 1.0012x over previous
"""Causal multi-head attention (B=2, T=2048, D=1024, H=16) on 8 TRN2 NeuronCores.

Strategy (tensor-parallel over heads + sequence-parallel output projection):
  - Each core owns 2 heads (e-slice of 128 columns of Q/K/V) for BOTH batches.
  - Per core: Q^T/K^T/V^T projections from a replicated transposed input x^T,
    flash-style causal attention computed entirely in the "transposed" layout
    (S^T chunks with k on partitions), softmax without max-subtraction
    (|S/8| < ~15 so exp is safe in fp32/bf16), row sums via a ones-column
    appended to V in the P^T.V matmul.
  - Two AllToAlls over all 8 cores (one per local head, the first overlapped
    with the second head's attention) reshard ctx from head-split to row-split
    (each core ends with ctx^T [1024, 512] for its (batch, seq-quarter)).
  - Output projection out[rows, :] = ctx rows @ w_o^T locally per core.
  - Host gathers the 8 disjoint [512, 1024] row blocks.
All matmuls in bf16 (fp32 PSUM accumulation).
"""

import numpy as np
import ml_dtypes

B, T, D, H = 2, 2048, 1024, 16
DH = D // H            # 64
NCORES = 8
ES = 128               # columns of Q/K/V per core (2 heads)
TQ = 512               # q-super width
NTQ = T // TQ          # 4 q-supers per (b, h)
KC = 128               # k-chunk width
NKC = T // KC          # 16 k-chunks
NDC = D // 128         # 8 contraction chunks

_cache = {}

PS_S_BUFS = 3
PS_PV_BUFS = 2
PS_MM_BUFS = 2
PS_MISC_BUFS = 1
PT_BUFS = 8
STAGE_BUFS = 4


def _build(reps=1):
    import concourse.bacc as bacc
    import concourse.mybir as mybir
    import concourse.tile as tile

    dt = mybir.dt
    fp32 = dt.float32
    bf16 = dt.bfloat16

    nc = bacc.Bacc("TRN2", target_bir_lowering=False, debug=False,
                   enable_asserts=False, num_devices=NCORES)

    xT_d = nc.dram_tensor("xT", [B, D, T], bf16, kind="ExternalInput")
    wqT_d = nc.dram_tensor("wqT", [D, ES], bf16, kind="ExternalInput")
    wkT_d = nc.dram_tensor("wkT", [D, ES], bf16, kind="ExternalInput")
    wvT_d = nc.dram_tensor("wvT", [D, ES], bf16, kind="ExternalInput")
    woT_d = nc.dram_tensor("woT", [D, D], bf16, kind="ExternalInput")
    masks_d = nc.dram_tensor("masks", [4, KC, TQ], bf16, kind="ExternalInput")
    ident_d = nc.dram_tensor("ident", [128, 128], bf16, kind="ExternalInput")
    out_d = nc.dram_tensor("out", [TQ, D], fp32, kind="ExternalOutput")

    with tile.TileContext(nc) as tc:
        with (
            tc.tile_pool(name="xt", bufs=B * NDC) as xt_pool,
            tc.tile_pool(name="wt", bufs=1) as wt_pool,
            tc.tile_pool(name="qkv", bufs=1) as qkv_pool,
            tc.tile_pool(name="pt", bufs=PT_BUFS) as pt_pool,
            tc.tile_pool(name="sm", bufs=4) as sm_pool,
            tc.tile_pool(name="stage", bufs=STAGE_BUFS) as stage_pool,
            tc.tile_pool(name="outp", bufs=3) as out_pool,
            tc.tile_pool(name="ps_s", bufs=PS_S_BUFS, space="PSUM") as ps_s,
            tc.tile_pool(name="ps_pv", bufs=PS_PV_BUFS, space="PSUM") as ps_pv,
            tc.tile_pool(name="ps_mm", bufs=PS_MM_BUFS, space="PSUM") as ps_mm,
            tc.tile_pool(name="ps_misc", bufs=PS_MISC_BUFS, space="PSUM") as ps_misc,
            tc.tile_pool(name="dram", bufs=2, space="DRAM") as dram_pool,
        ):
            for rep in range(reps):
                _emit(nc, tc, mybir, fp32, bf16,
                      xT_d, wqT_d, wkT_d, wvT_d, woT_d, masks_d, ident_d, out_d,
                      xt_pool, wt_pool, qkv_pool, pt_pool, sm_pool, stage_pool,
                      out_pool, ps_s, ps_pv, ps_mm, ps_misc, dram_pool)

    nc.compile()
    return nc


def _emit(nc, tc, mybir, fp32, bf16,
          xT_d, wqT_d, wkT_d, wvT_d, woT_d, masks_d, ident_d, out_d,
          xt_pool, wt_pool, qkv_pool, pt_pool, sm_pool, stage_pool,
          out_pool, ps_s, ps_pv, ps_mm, ps_misc, dram_pool):
    # ---- constant / weight loads ----
    wq_sb = wt_pool.tile([128, NDC, ES], bf16, name="wq_sb", tag="wq")
    wk_sb = wt_pool.tile([128, NDC, ES], bf16, name="wk_sb", tag="wk")
    wv_sb = wt_pool.tile([128, NDC, ES], bf16, name="wv_sb", tag="wv")
    nc.sync.dma_start(wq_sb[:], wqT_d.rearrange("(c p) e -> p c e", p=128))
    nc.sync.dma_start(wk_sb[:], wkT_d.rearrange("(c p) e -> p c e", p=128))
    nc.sync.dma_start(wv_sb[:], wvT_d.rearrange("(c p) e -> p c e", p=128))
    wo_sb = wt_pool.tile([128, NDC, D], bf16, name="wo_sb", tag="wo")
    mask_sb = wt_pool.tile([128, 4, TQ], bf16, name="mask_sb", tag="mask")
    ident_sb = wt_pool.tile([128, 128], bf16, name="ident_sb", tag="ident")
    ones_sb = wt_pool.tile([1, 64], bf16, name="ones_sb", tag="ones")
    nc.vector.memset(ones_sb[:], 1.0)

    # ---- per-batch pipeline: xT DMA -> projections (V transposed inline)
    # -> h=0 attention, so batch 1's projections overlap batch 0's attention.
    qt_sb = qkv_pool.tile([128, B, T], bf16, name="qt_sb", tag="qt")
    kt_sb = qkv_pool.tile([128, B, T], bf16, name="kt_sb", tag="kt")
    vt_sb = qkv_pool.tile([128, B, T], bf16, name="vt_sb", tag="vt")
    v_sb = qkv_pool.tile([128, B, NKC, 2 * (DH + 1)], bf16, name="v_sb", tag="v")
    nc.vector.memset(v_sb[:], 1.0)

    # a2a buffer layout per h: [64 rows, 4096 bf16]; shard j = 8 rows.
    # partition p (0..63) of the [64, 512] (b, s)-chunk lives at
    # row 8*j_dst + p//8, col (p%8)*512, where j_dst = 4b + s.
    a2a_in = {}
    a2a_out = {}
    for h in range(2):
        a2a_in[h] = dram_pool.tile([64, 4096], bf16, name=f"a2a_in{h}",
                                   tag=f"a2a_in{h}")
        a2a_out[h] = dram_pool.tile([64, 4096], bf16, name=f"a2a_out{h}",
                                    tag=f"a2a_out{h}")

    xt = {}

    def emit_xt(b):
        # x^T tiles [128, T] per d-chunk; DMA in half-tiles, half-major, so
        # the first t-supers' projections start after ~2MB instead of ~4MB
        for c in range(NDC):
            xt[(b, c)] = xt_pool.tile([128, T], bf16, name=f"xt_{b}_{c}",
                                      tag="xt")
        for half in range(2):
            for c in range(NDC):
                nc.sync.dma_start(
                    xt[(b, c)][:, 1024 * half: 1024 * (half + 1)],
                    xT_d[b, 128 * c:128 * (c + 1),
                         1024 * half: 1024 * (half + 1)])

    def emit_proj(b, s):
        for w_sb, dst in ((wq_sb, qt_sb), (wk_sb, kt_sb), (wv_sb, vt_sb)):
            mm = ps_mm.tile([128, TQ], fp32, name="mm", tag="mm")
            for c in range(NDC):
                nc.tensor.matmul(
                    mm[:], w_sb[:, c, :],
                    xt[(b, c)][:, TQ * s: TQ * (s + 1)],
                    start=(c == 0), stop=(c == NDC - 1))
            nc.vector.tensor_copy(dst[:, b, TQ * s: TQ * (s + 1)], mm[:])
        # V natural for this t-super's 4 k-chunks, with ones columns
        for i in range(4 * s, 4 * s + 4):
            tp = ps_misc.tile([128, 128], bf16, name="tp", tag="misc")
            nc.tensor.transpose(
                tp[:], vt_sb[:, b, 128 * i: 128 * (i + 1)], ident_sb[:])
            nc.vector.tensor_copy(
                v_sb[:, b, i, :].rearrange("p (h e) -> p h e", h=2)[:, :, 0:DH],
                tp.rearrange("p (h e) -> p h e", h=2))

    def emit_attn(h, b, s_list=None):
        for s in (range(NTQ) if s_list is None else s_list):
            nj = 4 * s + 4
            pv = ps_pv.tile([DH + 1, TQ], fp32, name="pv", tag="pv")
            for j in range(nj):
                v_ = j - 4 * s
                q0 = 128 * v_ if v_ > 0 else 0   # valid q starts here
                sp = ps_s.tile([128, TQ], fp32, name="sp", tag="sp")
                nc.tensor.matmul(
                    sp[:, q0:],
                    kt_sb[DH * h: DH * (h + 1), b, KC * j: KC * (j + 1)],
                    qt_sb[DH * h: DH * (h + 1), b, TQ * s + q0: TQ * (s + 1)],
                    start=True, stop=True)
                pt = pt_pool.tile([128, TQ], bf16, name="pt", tag="pt")
                nc.scalar.activation(
                    pt[:, q0:], sp[:, q0:],
                    mybir.ActivationFunctionType.Exp, scale=0.125)
                if v_ >= 0:
                    nc.vector.tensor_mul(
                        pt[:, q0:], pt[:, q0:], mask_sb[:, v_, q0:])
                nc.tensor.matmul(
                    pv[:, q0:],
                    v_sb[:, b, j, (DH + 1) * h: (DH + 1) * (h + 1)],
                    pt[:, q0:],
                    start=(j == 0), stop=(j == nj - 1))
            # normalize: linv = 1/l broadcast over 64 partitions via PE
            linv_f = sm_pool.tile([1, TQ], fp32, name="linv_f", tag="lf")
            nc.vector.reciprocal(linv_f[:], pv[DH:DH + 1, :])
            linv_b = sm_pool.tile([1, TQ], bf16, name="linv_b", tag="lb")
            nc.vector.tensor_copy(linv_b[:], linv_f[:])
            bc = ps_misc.tile([DH, TQ], fp32, name="bc", tag="misc")
            nc.tensor.matmul(bc[:], ones_sb[:], linv_b[:],
                             start=True, stop=True)
            bcb = sm_pool.tile([DH, TQ], fp32, name="bcb", tag="bcb")
            nc.vector.tensor_copy(bcb[:], bc[:])
            ctx = stage_pool.tile([DH, TQ], bf16, name="ctx", tag="ctx")
            nc.vector.tensor_mul(ctx[:], pv[0:DH, :], bcb[:])
            j_dst = 4 * b + s
            r0 = 8 * j_dst
            nc.sync.dma_start(
                a2a_in[h][r0: r0 + 8, :].rearrange(
                    "r (s2 n) -> (r s2) n", s2=8),
                ctx[:])

    def emit_a2a(h):
        nc.gpsimd.collective_compute(
            "AllToAll", mybir.AluOpType.bypass,
            replica_groups=[list(range(NCORES))],
            ins=[a2a_in[h].opt()], outs=[a2a_out[h].opt()])

    # pipeline: interleave h=0 attention with projections at t-super
    # granularity so the exp stream starts as early as possible; batch-1
    # projections overlap batch-0 attention; first A2A (h=0 data) overlaps
    # second-head attention.
    emit_xt(0)
    nc.sync.dma_start(ident_sb[:], ident_d[:])
    nc.sync.dma_start(mask_sb[:], masks_d.rearrange("v p n -> p v n"))
    for s in range(NTQ):
        emit_proj(0, s)
        emit_attn(0, 0, [s])
    emit_xt(1)
    nc.sync.dma_start(wo_sb[:], woT_d.rearrange("(c p) e -> p c e", p=128))
    for s in range(NTQ):
        emit_proj(1, s)
        emit_attn(0, 1, [s])
    emit_a2a(0)
    emit_attn(1, 0)
    emit_attn(1, 1)

    # gather received shards: ctx^T full [1024, 512] as [128, 8, 512];
    # e-chunk j rows 0:64 = head 2j (h=0 a2a), rows 64:128 = head 2j+1.
    emit_a2a(1)
    ctxf = qkv_pool.tile([128, NDC, TQ], bf16, name="ctxf", tag="ctxf")
    for h in range(2):
        for j in range(NDC):
            nc.sync.dma_start(
                ctxf[64 * h: 64 * (h + 1), j, :],
                a2a_out[h][8 * j: 8 * (j + 1), :].rearrange(
                    "r (s2 n) -> (r s2) n", s2=8))

    # ---- output projection ----
    for i in range(TQ // 128):
        for n in range(2):
            mm = ps_mm.tile([128, TQ], fp32, name="mm", tag="mm")
            for j in range(NDC):
                nc.tensor.matmul(
                    mm[:], ctxf[:, j, 128 * i: 128 * (i + 1)],
                    wo_sb[:, j, TQ * n: TQ * (n + 1)],
                    start=(j == 0), stop=(j == NDC - 1))
            ob = out_pool.tile([128, TQ], fp32, name="ob", tag="ob")
            nc.vector.tensor_copy(ob[:], mm[:])
            nc.sync.dma_start(
                out_d[128 * i: 128 * (i + 1), TQ * n: TQ * (n + 1)], ob[:])


def _prep_inputs(x, w_q, w_k, w_v, w_o):
    bf = ml_dtypes.bfloat16
    xT = np.ascontiguousarray(np.transpose(np.asarray(x, np.float32),
                                           (0, 2, 1))).astype(bf)
    woT = np.ascontiguousarray(np.asarray(w_o, np.float32).T).astype(bf)
    masks = np.zeros((4, KC, TQ), dtype=bf)
    ql = np.arange(TQ)[None, :]
    kl = np.arange(KC)[:, None]
    for v in range(4):
        masks[v] = (ql >= kl + 128 * v).astype(bf)
    ident = np.eye(128, dtype=bf)
    in_maps = []
    for c in range(NCORES):
        # heads (2c, 2c+1): h=0 slice rows [128c, 128c+64), h=1 the next 64
        sl = slice(ES * c, ES * (c + 1))
        in_maps.append({
            "xT": xT,
            "wqT": np.ascontiguousarray(np.asarray(w_q, np.float32)[sl, :].T).astype(bf),
            "wkT": np.ascontiguousarray(np.asarray(w_k, np.float32)[sl, :].T).astype(bf),
            "wvT": np.ascontiguousarray(np.asarray(w_v, np.float32)[sl, :].T).astype(bf),
            "woT": woT,
            "masks": masks,
            "ident": ident,
        })
    return in_maps


def kernel(x, w_q, w_k, w_v, w_o, _run_kwargs=None):
    from concourse.bass_utils import run_bass_kernel_spmd
    if "nc" not in _cache:
        _cache["nc"] = _build()
    nc = _cache["nc"]
    in_maps = _prep_inputs(x, w_q, w_k, w_v, w_o)
    res = run_bass_kernel_spmd(nc, in_maps, core_ids=list(range(NCORES)),
                               **(_run_kwargs or {}))
    _cache["last_result"] = res
    out = np.empty((B, T, D), dtype=np.float32)
    for c in range(NCORES):
        b, s = c // 4, c % 4
        out[b, TQ * s: TQ * (s + 1), :] = res.results[c]["out"]
    return out
